# revision 1
# baseline (speedup 1.0000x reference)
"""Bass/Trainium2 kernel for the NaiveGNN message-passing problem.

Math (reference): h = emb @ W0 + b0 + sum_l (sum_j sigmoid(ee @ W1s[l])) @ W2s[l]
with ee[i,j] = [r_i - r_j, |r_i - r_j|^2].

Key identities used:
  - The three layers share the ee tensor, so W1s concatenates to W1cat [4,96]
    and W2s to W2cat [96,64]: one fused pass with H=96 sigmoid features.
  - ee @ W1cat decomposes through the Gram matrix G = r @ r^T:
        Z[i,j,h] = A[i,h] + B[j,h] + s_h * G[i,j],   s_h = -2*W1cat[3,h]
        A[i,h] =  r_i . w_h + |r_i|^2 * w4_h
        B[j,h] = -r_j . w_h + |r_j|^2 * w4_h
    so the [E,E,4] pairwise tensor is never materialized. Per (h, i-tile) the
    tensor engine computes T_h = lhsT.T @ rhs into PSUM with
    lhsT = [s_h*r_i ; 1] (4 x 128) and rhs = [r_j ; B_h] (4 x 2048), and the
    scalar engine applies sigmoid with per-partition bias A[:,h], accumulating
    over the free (j) axis directly into S[:,h] via accum_out.

Sharding: i-axis split across 8 cores (256 rows each); every core holds the
full r for the j axis, no collectives. Output slices are concatenated on host.
"""

import numpy as np

E = 2048
NCORES = 8
EI = E // NCORES  # 256 rows per core
H = 96
NNUC = 64

_CACHE = {}


def _split_sync_waits(bir_json):
    """This walrus build accepts at most ONE sync wait per instruction
    (setupSyncWait: 'Too many sync wait commands'), while Tile freely attaches
    several. Rewrite the BIR: move all but one wait of each instruction onto
    single-wait NoOps on the same engine immediately before it — the engine's
    in-order sequencer makes this semantically identical."""
    import json

    m = json.loads(bir_json)
    ctr = 0
    for fn in m["functions"]:
        for blk in fn["blocks"]:
            out = []
            for inst in blk["instructions"]:
                si = inst.get("sync_info")
                waits = (si or {}).get("on_wait") or []
                if len(waits) > 1:
                    for w in waits[:-1]:
                        ctr += 1
                        out.append(
                            {
                                "debug": inst.get("debug", 0),
                                "engine": inst["engine"],
                                "ins": [],
                                "name": f"WSPLIT-{ctr}",
                                "opcode": "NoOp",
                                "outs": [],
                                "sync_info": {"on_update": [], "on_wait": [w]},
                            }
                        )
                    si["on_wait"] = [waits[-1]]
                out.append(inst)
            blk["instructions"] = out
    return json.dumps(m).encode()


def _install_compile_patch():
    if _CACHE.get("patched"):
        return
    import concourse.bass_utils as bu
    import concourse.bass2jax as b2j

    orig = bu.compile_bir_kernel

    def patched(bir_json, tmpdir, neff_name="file.neff"):
        return orig(_split_sync_waits(bir_json), tmpdir, neff_name)

    bu.compile_bir_kernel = patched
    b2j.compile_bir_kernel = patched
    _CACHE["patched"] = True


def _build():
    import concourse.bass as bass
    import concourse.tile as tile
    from concourse import mybir
    from concourse.vector_clock import ScopedClock, VectorClock

    f32 = mybir.dt.float32
    f16 = mybir.dt.float16
    AF = mybir.ActivationFunctionType

    class _TC(tile.TileContext):
        # This walrus build rejects instructions carrying more than ~2 sem
        # waits; the stock tail drain carries one per logical processor.
        # Split them into single-wait NOPs on the sync engine ahead of it.
        def _drain_and_barrier(self, tick_clock, wait_clock):
            gc = tick_clock.global_clock
            n = len(gc)
            for p in range(n):
                t = gc[p]
                if t > 0:
                    vec = [0] * n
                    vec[p] = t
                    nop = self.nc.sync.nop()
                    wait_clock.add_sem_waits(
                        nop.ins, ScopedClock({None: VectorClock(vec)})
                    )
            self.nc.sync.drain()
            self.nc.all_engine_barrier()
            popped = self.nc._tile_sem_poison_stack.pop()
            assert popped is self._sem_poison
            self.nc.clear_and_free_semaphores(list(self.sems.allocated().values()))
            self.nc.all_engine_barrier()

    nc = bass.Bass(name="gnn")
    U5 = nc.dram_tensor("U5", [5, E], f32, kind="ExternalInput")
    den = nc.dram_tensor("den", [3 * NNUC, EI], f32, kind="ExternalInput")
    spin1 = nc.dram_tensor("spin1", [2, EI], f32, kind="ExternalInput")
    BTd = nc.dram_tensor("BTd", [H, E], f16, kind="ExternalInput")
    Ain = nc.dram_tensor("Ain", [EI, H], f32, kind="ExternalInput")
    S3 = nc.dram_tensor("S3", [3, H], f32, kind="ExternalInput")
    REN = nc.dram_tensor("REN", [5, NNUC], f32, kind="ExternalInput")
    W2A = nc.dram_tensor("W2A", [H, 64], f32, kind="ExternalInput")
    W0A = nc.dram_tensor("W0A", [128, 64], f32, kind="ExternalInput")
    W0B = nc.dram_tensor("W0B", [128, 64], f32, kind="ExternalInput")
    W0C = nc.dram_tensor("W0C", [2, 64], f32, kind="ExternalInput")
    EYE = nc.dram_tensor("EYE", [128, 128], f32, kind="ExternalInput")
    ONES = nc.dram_tensor("ONES", [1, 128], f16, kind="ExternalInput")
    U5L = nc.dram_tensor("U5L", [5, EI], f32, kind="ExternalInput")
    out = nc.dram_tensor("out", [EI, 64], f32, kind="ExternalOutput")

    with _TC(nc) as tc:
        import contextlib

        with contextlib.ExitStack() as ctx:
            const = ctx.enter_context(tc.tile_pool(name="const", bufs=1))
            work = ctx.enter_context(tc.tile_pool(name="work", bufs=2))
            psum = ctx.enter_context(tc.tile_pool(name="psum", bufs=2, space="PSUM"))

            def load(dram, shape, name):
                t = const.tile(shape, f32, tag=name)
                nc.sync.dma_start(out=t, in_=dram[:, :])
                return t

            U5_sb = load(U5, [5, E], "U5")
            den_sb = const.tile([128, EI], f32, tag="den_hi", name="den_hi")  # k=0,1 blocks
            nc.sync.dma_start(out=den_sb, in_=den[0:128, :])
            denb_sb = const.tile([64, EI], f32, tag="den_lo", name="den_lo")  # k=2 block
            nc.sync.dma_start(out=denb_sb, in_=den[128:192, :])
            spin1_sb = load(spin1, [2, EI], "spin1")
            S3_sb = load(S3, [3, H], "S3")
            REN_sb = load(REN, [5, NNUC], "REN")
            W2A_sb = load(W2A, [H, 64], "W2A")
            W0A_sb = load(W0A, [128, 64], "W0A")
            W0B_sb = load(W0B, [128, 64], "W0B")
            W0C_sb = load(W0C, [2, 64], "W0C")
            EYE_sb = load(EYE, [128, 128], "EYE")
            U5L_sb = load(U5L, [5, EI], "U5L")

            A_sb = []
            S_sb = []
            embA_sb = []
            embB_sb = []
            dist_sb = []
            logd_sb = []
            for t in range(2):
                A_sb.append(const.tile([128, H], f32, tag=f"A{t}", name=f"A{t}"))
                S_sb.append(const.tile([128, H], f32, tag=f"S{t}", name=f"S{t}"))
                embA_sb.append(const.tile([128, 128], f32, tag=f"embA{t}", name=f"embA{t}"))
                embB_sb.append(const.tile([128, 128], f32, tag=f"embB{t}", name=f"embB{t}"))

            for t in range(2):
                isl = slice(t * 128, (t + 1) * 128)
                nc.sync.dma_start(out=A_sb[t], in_=Ain[t * 128 : (t + 1) * 128, :])
                # dist^2 -> dist
                D2_ps = psum.tile([128, NNUC], f32, tag="ps", name="ps")
                nc.tensor.matmul(D2_ps, U5L_sb[0:5, isl], REN_sb, start=True, stop=True)
                d_t = work.tile([128, NNUC], f32, tag="dist", name="dist")
                nc.scalar.activation(out=d_t, in_=D2_ps, func=AF.Sqrt)
                dist_sb.append(d_t)
            for t in range(2):
                l_t = work.tile([128, NNUC], f32, tag="logd", name="logd")
                nc.scalar.activation(out=l_t, in_=dist_sb[t], func=AF.Ln, bias=1.0)
                logd_sb.append(l_t)
            for t in range(2):
                isl = slice(t * 128, (t + 1) * 128)
                rec = work.tile([128, NNUC], f32, tag="rec", name="rec")
                nc.vector.reciprocal(rec, dist_sb[t])
                # stack [g | g] and [logd | logd] so the transposed copies
                # exist on both partition halves (DVE lanes cannot cross
                # partitions, so each consumer needs an aligned source)
                g2 = work.tile([128, 128], f32, tag="g2", name="g2")
                nc.vector.tensor_mul(g2[:, 0:NNUC], logd_sb[t], rec)
                nc.vector.tensor_copy(g2[:, NNUC:128], g2[:, 0:NNUC])
                l2 = work.tile([128, 128], f32, tag="l2", name="l2")
                nc.vector.tensor_copy(l2[:, 0:NNUC], logd_sb[t])
                nc.vector.tensor_copy(l2[:, NNUC:128], logd_sb[t])
                g2T_ps = psum.tile([128, 128], f32, tag="ps", name="ps")
                nc.tensor.transpose(g2T_ps, g2, EYE_sb)
                g2T = work.tile([128, 128], f32, tag="g2T", name="g2T")
                nc.vector.tensor_copy(g2T, g2T_ps)
                l2T_ps = psum.tile([128, 128], f32, tag="ps", name="ps")
                nc.tensor.transpose(l2T_ps, l2, EYE_sb)
                # embT chunk A: k=0 block rows 0:64, k=1 block rows 64:128
                nc.vector.tensor_mul(embA_sb[t][0:64, :], den_sb[0:64, isl], g2T[0:64, :])
                nc.vector.tensor_mul(embA_sb[t][64:128, :], den_sb[64:128, isl], g2T[64:128, :])
                # embT chunk B: k=2 block rows 0:64, log-dist rows 64:128
                nc.vector.tensor_mul(embB_sb[t][0:64, :], denb_sb[:, isl], g2T[0:64, :])
                nc.vector.tensor_copy(embB_sb[t][64:128, :], l2T_ps[64:128, :])

            # rotating rhs buffers V[b] = [r_j(3) ; B_h(1)] and lhsT buffers
            NBUF = 3
            V = []
            for b in range(NBUF):
                v = const.tile([4, E], f16, tag=f"V{b}", name=f"V{b}")
                nc.vector.tensor_copy(v[0:3, :], U5_sb[0:3, :])
                V.append(v)
            L = []
            for t in range(2):
                row = []
                for b in range(NBUF):
                    lb = const.tile([4, 128], f16, tag=f"L{t}{b}", name=f"L{t}{b}")
                    nc.sync.dma_start(out=lb[3:4, :], in_=ONES[:, :])
                    row.append(lb)
                L.append(row)

            for h in range(H):
                b = h % NBUF
                nc.sync.dma_start(out=V[b][3:4, :], in_=BTd[h : h + 1, :])
                for t in range(2):
                    isl = slice(t * 128, (t + 1) * 128)
                    lb = L[t][b]
                    nc.vector.tensor_scalar_mul(
                        lb[0:3, :], U5L_sb[0:3, isl], S3_sb[0:3, h : h + 1]
                    )
                    T_ps = psum.tile([128, E], f32, tag="ps", name="ps")
                    for c in range(4):
                        nc.tensor.matmul(
                            T_ps[:, c * 512 : (c + 1) * 512],
                            lb,
                            V[b][:, c * 512 : (c + 1) * 512],
                            start=True,
                            stop=True,
                        )
                    nc.scalar.activation(
                        out=T_ps,
                        in_=T_ps,
                        func=AF.Sigmoid,
                        bias=A_sb[t][:, h : h + 1],
                        accum_out=S_sb[t][:, h : h + 1],
                    )

            for t in range(2):
                isl = slice(t * 128, (t + 1) * 128)
                ST_ps = psum.tile([H, 128], f32, tag="ps", name="ps")
                nc.tensor.transpose(ST_ps, S_sb[t], EYE_sb)
                ST_sb = work.tile([H, 128], f32, tag="ST", name="ST")
                nc.vector.tensor_copy(ST_sb, ST_ps)
                O_ps = psum.tile([128, 64], f32, tag="ps", name="ps")
                nc.tensor.matmul(O_ps, ST_sb, W2A_sb, start=True, stop=False)
                nc.tensor.matmul(O_ps, embA_sb[t], W0A_sb, start=False, stop=False)
                nc.tensor.matmul(O_ps, embB_sb[t], W0B_sb, start=False, stop=False)
                nc.tensor.matmul(
                    O_ps, spin1_sb[:, isl], W0C_sb, start=False, stop=True
                )
                O_sb = work.tile([128, 64], f32, tag="O", name="O")
                nc.vector.tensor_copy(O_sb, O_ps)
                nc.sync.dma_start(out=out[isl, :], in_=O_sb)

    return nc


def _host_prep(r, R, W0, b0, W1s, W2s, n_up, n_down):
    r = np.asarray(r, np.float32)
    R = np.asarray(R, np.float32)
    W0 = np.asarray(W0, np.float32)
    b0 = np.asarray(b0, np.float32)
    W1s = np.asarray(W1s, np.float32)
    W2s = np.asarray(W2s, np.float32)
    n_up = int(n_up)

    W1cat = np.concatenate([W1s[0], W1s[1], W1s[2]], axis=1)  # [4, 96]
    w4 = W1cat[3]
    S3 = np.broadcast_to(-2.0 * w4, (3, H)).astype(np.float32).copy()
    W2cat = np.concatenate([W2s[0], W2s[1], W2s[2]], axis=0).astype(np.float32)

    n2 = (r * r).sum(1).astype(np.float32)
    # A[i,h] = r_i.w_h + |r_i|^2 w4_h ; B[j,h] = -r_j.w_h + |r_j|^2 w4_h
    rw = r @ W1cat[0:3]
    n2w4 = n2[:, None] * w4[None, :]
    Afull = (rw + n2w4).astype(np.float32)          # [E, H]
    BT = np.ascontiguousarray((-rw + n2w4).T.astype(np.float16))  # [H, E]
    U5 = np.stack(
        [r[:, 0], r[:, 1], r[:, 2], n2, np.ones(E, np.float32)]
    ).astype(np.float32)

    R2 = (R * R).sum(1).astype(np.float32)
    REN = np.concatenate(
        [-2.0 * R.T, np.ones((1, NNUC), np.float32), R2[None]], axis=0
    ).astype(np.float32)

    # den[k*64+n, i] = r[i,k] - R[n,k]
    den = (r.T[:, None, :] - R.T[:, :, None]).reshape(3 * NNUC, E).astype(np.float32)

    spin = np.ones(E, np.float32)
    spin[n_up:] = -1.0
    spin1 = np.stack([spin, np.ones(E, np.float32)]).astype(np.float32)

    # permuted W0 chunks matching the on-device embT feature order
    n_idx = np.arange(NNUC)
    perm_a = np.concatenate([3 * n_idx, 3 * n_idx + 1])
    perm_b = np.concatenate([3 * n_idx + 2, 192 + n_idx])
    W0A = W0[perm_a].astype(np.float32)
    W0B = W0[perm_b].astype(np.float32)
    W0C = np.stack([W0[256], b0]).astype(np.float32)

    eye = np.eye(128, dtype=np.float32)

    shared = {
        "U5": U5,
        "BTd": BT,
        "S3": S3,
        "REN": REN,
        "W2A": W2cat,
        "W0A": W0A,
        "W0B": W0B,
        "W0C": W0C,
        "EYE": eye,
        "ONES": np.ones((1, 128), np.float16),
    }
    in_maps = []
    for c in range(NCORES):
        isl = slice(c * EI, (c + 1) * EI)
        m = dict(shared)
        m["den"] = np.ascontiguousarray(den[:, isl])
        m["spin1"] = np.ascontiguousarray(spin1[:, isl])
        m["U5L"] = np.ascontiguousarray(U5[:, isl])
        m["Ain"] = np.ascontiguousarray(Afull[isl, :])
        in_maps.append(m)
    return in_maps


def _get_runner():
    """Build the Bass program once and hold a single jitted shard_map
    executable so repeat kernel() calls skip retracing/recompiling.
    Mirrors concourse.bass2jax.run_bass_via_pjrt's multi-core path."""
    if "runner" in _CACHE:
        return _CACHE["runner"]

    import jax
    from jax.experimental.shard_map import shard_map
    from jax.sharding import Mesh, PartitionSpec

    from concourse import mybir
    from concourse.bass2jax import (
        _bass_exec_p,
        install_neuronx_cc_hook,
        partition_id_tensor,
    )

    _install_compile_patch()
    install_neuronx_cc_hook()
    nc = _CACHE.setdefault("nc", _build())

    partition_name = nc.partition_id_tensor.name if nc.partition_id_tensor else None
    in_names = []
    out_names = []
    out_avals = []
    zero_outs = []
    for alloc in nc.m.functions[0].allocations:
        if not isinstance(alloc, mybir.MemoryLocationSet):
            continue
        name = alloc.memorylocations[0].name
        if alloc.kind == "ExternalInput":
            if name != partition_name:
                in_names.append(name)
        elif alloc.kind == "ExternalOutput":
            shape = tuple(alloc.tensor_shape)
            dtype = mybir.dt.np(alloc.dtype)
            out_names.append(name)
            out_avals.append(jax.core.ShapedArray(shape, dtype))
            zero_outs.append(np.zeros(shape, dtype))
    n_params = len(in_names)
    n_outs = len(out_names)
    all_in_names = list(in_names) + list(out_names)
    if partition_name is not None:
        all_in_names.append(partition_name)
    donate = tuple(range(n_params, n_params + n_outs))

    def _body(*args):
        operands = list(args)
        if partition_name is not None:
            operands.append(partition_id_tensor())
        outs = _bass_exec_p.bind(
            *operands,
            out_avals=tuple(out_avals),
            in_names=tuple(all_in_names),
            out_names=tuple(out_names),
            lowering_input_output_aliases=(),
            sim_require_finite=True,
            sim_require_nnan=True,
            nc=nc,
        )
        return tuple(outs)

    devices = jax.devices()[:NCORES]
    mesh = Mesh(np.asarray(devices), ("core",))
    in_specs = (PartitionSpec("core"),) * (n_params + n_outs)
    out_specs = (PartitionSpec("core"),) * n_outs
    sharded = jax.jit(
        shard_map(
            _body, mesh=mesh, in_specs=in_specs, out_specs=out_specs, check_rep=False
        ),
        donate_argnums=donate,
        keep_unused=True,
    )

    def runner(in_maps):
        concat_in = [
            np.concatenate([np.asarray(in_maps[c][n]) for c in range(NCORES)], axis=0)
            for n in in_names
        ]
        concat_zeros = [
            np.zeros((NCORES * z.shape[0], *z.shape[1:]), z.dtype) for z in zero_outs
        ]
        out_arrs = sharded(*concat_in, *concat_zeros)
        return np.asarray(out_arrs[out_names.index("out")])

    _CACHE["runner"] = runner
    return runner


def kernel(r, R, W0, b0, W1s, W2s, n_up, n_down):
    runner = _get_runner()
    in_maps = _host_prep(r, R, W0, b0, W1s, W2s, n_up, n_down)
    return runner(in_maps)



# revision 5
# speedup vs baseline: 1.2061x; 1.2061x over previous
"""Bass/Trainium2 kernel for the NaiveGNN message-passing problem.

Math (reference): h = emb @ W0 + b0 + sum_l (sum_j sigmoid(ee @ W1s[l])) @ W2s[l]
with ee[i,j] = [r_i - r_j, |r_i - r_j|^2].

Key identities used:
  - The three layers share the ee tensor, so W1s concatenates to W1cat [4,96]
    and W2s to W2cat [96,64]: one fused pass with H=96 sigmoid features.
  - ee @ W1cat decomposes through the Gram matrix G = r @ r^T:
        Z[i,j,h] = A[i,h] + B[j,h] + s_h * G[i,j],   s_h = -2*W1cat[3,h]
    so the [E,E,4] pairwise tensor is never materialized. Per (h, i-tile) the
    tensor engine computes y = a2*Z + gam into PSUM with a K=5 matmul
    (lhsT = [a2*s_h*r_i ; a2 ; a2*A+gam], rhs = [r_j ; B_h ; 1]).
  - The sigmoid+sum over j is SPLIT across two engines per (h, i-tile):
      * ScalarE (ACT): exact sigmoid on j in [0,1024): sigmoid(y/a2 - gam/a2)
        with accum_out accumulating into S.
      * VectorE (DVE): a custom 8-stage fused op on j in [1024,2048) computing
        a 5-segment piecewise-linear sigmoid approximation
            clip(max(min(rho*y, y+c1), y+c2), 0, 1)
        with a fused add-reduction (single pass, 1 elem/cycle/lane).
    This roughly halves the ScalarE bottleneck (the baseline ran all 96*2*2048
    sigmoid columns through ACT alone). The PL approximation error (max 0.017
    per element) is mean-zero-ish across the j-sum; end-to-end output rel err
    stays ~1e-3, well under the 2e-2 gate.

Sharding: i-axis split across 8 cores (256 rows each); every core holds the
full r for the j axis, no collectives. Output slices are concatenated on host.
"""

import numpy as np

E = 2048
NCORES = 8
EI = E // NCORES  # 256 rows per core
H = 96
NNUC = 64
NU = 2 * H  # (h, i-tile) units per core
JH = 1024  # j-range handled per engine per unit

# 5-segment piecewise-linear sigmoid fit (minimax, max err 0.0174):
#   clip(max(min(a1*z + 0.5, a2*z + 0.5 + c), a2*z + 0.5 - c), 0, 1)
# a2 is rounded to fp16 so the PE-folded scale matches the host fold exactly.
_A1 = 0.21579171
_A2 = float(np.float16(0.06016919))
_CC = 0.25722502
_RHO = _A1 / _A2
_GAM = 0.5 / _RHO
_PC1 = 0.5 + _CC - _GAM
_PC2 = 0.5 - _CC - _GAM
_SCL = 1.0 / _A2  # ACT: z = y*SCL + BIA recovers z from y = a2*z + gam
_BIA = -_GAM / _A2

_CACHE = {}


def _register_dve_op():
    """Register a custom DVE op computing the 5-seg PL sigmoid with fused
    add-reduction:
        out      = clip(max(min(in0*s0, in0+s1), in0+imm2), 0, 1)
        accum    = sum(out, axis=-1)
    Exactly 8 ALU stages incl. the accumulate; runs 1 elem/cycle/lane."""
    if "dve_op" in _CACHE:
        return _CACHE["dve_op"]
    from operator import add

    import concourse.dve_ops as dops
    from concourse.dve_spec import C0, C1, C2, One, Spec, Src0, Zero, lower, maxx, minn
    from concourse.dve_uop import DveOpSpec

    name = "SIGMOID_PL5_ANT"
    body = maxx(minn(maxx(minn(Src0 * C0, Src0 + C1), Src0 + C2), One), Zero)

    def ref(in0, in1, s0, s1, imm2):
        b = np.clip(
            np.maximum(np.minimum(in0.astype(np.float32) * s0, in0 + s1), in0 + imm2),
            0.0,
            1.0,
        ).astype(np.float32)
        return b, b.reshape(b.shape[0], -1).sum(axis=-1, keepdims=True)

    spec = Spec(body=body, accum=add, accum_init=Zero, reference=ref)
    row = dops._CUSTOM_DVE_ROW_BASE + len(dops.OPS)
    assert row < 0x20
    shas = {}
    for ver in ("v3", "v4"):
        s = DveOpSpec(name=name, opcode=row, uops=lower(spec, ver=ver), rd1_en=False)
        shas[ver] = s.sha(ver)
    op = dops.DveOp(name, spec, subdim=False, uops_sha=shas)
    dops.OPS.append(op)
    dops.CUSTOM_DVE_SPECS[name] = spec
    dops._SUB_OPCODE_FOR_NAME[name] = row
    _CACHE["dve_op"] = op
    return op


def _split_sync_waits(bir_json):
    """This walrus build accepts at most ONE sync wait per instruction
    (setupSyncWait: 'Too many sync wait commands'), while Tile freely attaches
    several. Rewrite the BIR: move all but one wait of each instruction onto
    single-wait NoOps on the same engine immediately before it — the engine's
    in-order sequencer makes this semantically identical."""
    import json

    m = json.loads(bir_json)
    ctr = 0
    for fn in m["functions"]:
        for blk in fn["blocks"]:
            out = []
            for inst in blk["instructions"]:
                si = inst.get("sync_info")
                waits = (si or {}).get("on_wait") or []
                if len(waits) > 1:
                    for w in waits[:-1]:
                        ctr += 1
                        out.append(
                            {
                                "debug": inst.get("debug", 0),
                                "engine": inst["engine"],
                                "ins": [],
                                "name": f"WSPLIT-{ctr}",
                                "opcode": "NoOp",
                                "outs": [],
                                "sync_info": {"on_update": [], "on_wait": [w]},
                            }
                        )
                    si["on_wait"] = [waits[-1]]
                out.append(inst)
            blk["instructions"] = out
    return json.dumps(m).encode()


def _install_compile_patch():
    if _CACHE.get("patched"):
        return
    import concourse.bass_utils as bu
    import concourse.bass2jax as b2j

    orig = bu.compile_bir_kernel

    def patched(bir_json, tmpdir, neff_name="file.neff"):
        return orig(_split_sync_waits(bir_json), tmpdir, neff_name)

    bu.compile_bir_kernel = patched
    b2j.compile_bir_kernel = patched
    _CACHE["patched"] = True


def _build():
    import contextlib

    import concourse.bass as bass
    import concourse.tile as tile
    from concourse import mybir
    from concourse.vector_clock import ScopedClock, VectorClock

    f32 = mybir.dt.float32
    f16 = mybir.dt.float16
    AF = mybir.ActivationFunctionType
    pl5 = _register_dve_op()

    class _TC(tile.TileContext):
        # This walrus build rejects instructions carrying more than ~2 sem
        # waits; the stock tail drain carries one per logical processor.
        # Split them into single-wait NOPs on the sync engine ahead of it.
        def _drain_and_barrier(self, tick_clock, wait_clock):
            gc = tick_clock.global_clock
            n = len(gc)
            for p in range(n):
                t = gc[p]
                if t > 0:
                    vec = [0] * n
                    vec[p] = t
                    nop = self.nc.sync.nop()
                    wait_clock.add_sem_waits(
                        nop.ins, ScopedClock({None: VectorClock(vec)})
                    )
            self.nc.sync.drain()
            self.nc.all_engine_barrier()
            popped = self.nc._tile_sem_poison_stack.pop()
            assert popped is self._sem_poison
            self.nc.clear_and_free_semaphores(list(self.sems.allocated().values()))
            self.nc.all_engine_barrier()

    nc = bass.Bass(name="gnn")
    den = nc.dram_tensor("den", [3 * NNUC, EI], f32, kind="ExternalInput")
    spin1 = nc.dram_tensor("spin1", [2, EI], f32, kind="ExternalInput")
    BTd = nc.dram_tensor("BTd", [H, E], f16, kind="ExternalInput")
    RJ4 = nc.dram_tensor("RJ4", [4, E], f16, kind="ExternalInput")  # [r;ones]
    LTd = nc.dram_tensor("LTd", [5, NU * 128], f16, kind="ExternalInput")
    REN = nc.dram_tensor("REN", [5, NNUC], f32, kind="ExternalInput")
    W2A = nc.dram_tensor("W2A", [H, 64], f32, kind="ExternalInput")
    W0A = nc.dram_tensor("W0A", [128, 64], f32, kind="ExternalInput")
    W0B = nc.dram_tensor("W0B", [128, 64], f32, kind="ExternalInput")
    W0C = nc.dram_tensor("W0C", [2, 64], f32, kind="ExternalInput")
    EYE = nc.dram_tensor("EYE", [128, 128], f32, kind="ExternalInput")
    U5L = nc.dram_tensor("U5L", [5, EI], f32, kind="ExternalInput")
    out = nc.dram_tensor("out", [EI, 64], f32, kind="ExternalOutput")

    with _TC(nc) as tc:
        with contextlib.ExitStack() as ctx:
            const = ctx.enter_context(tc.tile_pool(name="const", bufs=1))
            work = ctx.enter_context(tc.tile_pool(name="work", bufs=2))

            def load(dram, shape, name):
                t = const.tile(shape, f32, tag=name)
                nc.sync.dma_start(out=t, in_=dram[:, :])
                return t

            den_sb = const.tile([128, EI], f32, tag="den_hi", name="den_hi")
            nc.sync.dma_start(out=den_sb, in_=den[0:128, :])
            denb_sb = const.tile([64, EI], f32, tag="den_lo", name="den_lo")
            nc.sync.dma_start(out=denb_sb, in_=den[128:192, :])
            spin1_sb = load(spin1, [2, EI], "spin1")
            REN_sb = load(REN, [5, NNUC], "REN")
            W2A_sb = load(W2A, [H, 64], "W2A")
            W0A_sb = load(W0A, [128, 64], "W0A")
            W0B_sb = load(W0B, [128, 64], "W0B")
            W0C_sb = load(W0C, [2, 64], "W0C")
            EYE_sb = load(EYE, [128, 128], "EYE")
            U5L_sb = load(U5L, [5, EI], "U5L")
            LT_sb = const.tile([5, NU * 128], f16, tag="LT", name="LT")
            nc.sync.dma_start(out=LT_sb, in_=LTd[:, :])
            bia_sb = const.tile([128, 1], f32, tag="bia", name="bia")
            nc.gpsimd.memset(bia_sb, _BIA)

            # Per-engine accumulator tiles: SA (ACT) and SD (DVE), per i-tile.
            SA_sb = []
            SD_sb = []
            S_sb = []
            embA_sb = []
            embB_sb = []
            dist_sb = []
            logd_sb = []
            for t in range(2):
                SA_sb.append(const.tile([128, H], f32, tag=f"SA{t}", name=f"SA{t}"))
                SD_sb.append(const.tile([128, H], f32, tag=f"SD{t}", name=f"SD{t}"))
                S_sb.append(const.tile([128, H], f32, tag=f"S{t}", name=f"S{t}"))
                embA_sb.append(
                    const.tile([128, 128], f32, tag=f"embA{t}", name=f"embA{t}")
                )
                embB_sb.append(
                    const.tile([128, 128], f32, tag=f"embB{t}", name=f"embB{t}")
                )

            # ---- head: electron-nucleus features (own PSUM scope) ----
            with tc.tile_pool(name="hpsum", bufs=2, space="PSUM") as hpsum:
                for t in range(2):
                    isl = slice(t * 128, (t + 1) * 128)
                    D2_ps = hpsum.tile([128, NNUC], f32, tag="ps", name="ps")
                    nc.tensor.matmul(
                        D2_ps, U5L_sb[0:5, isl], REN_sb, start=True, stop=True
                    )
                    d_t = work.tile([128, NNUC], f32, tag="dist", name="dist")
                    nc.scalar.activation(out=d_t, in_=D2_ps, func=AF.Sqrt)
                    dist_sb.append(d_t)
                for t in range(2):
                    l_t = work.tile([128, NNUC], f32, tag="logd", name="logd")
                    nc.scalar.activation(out=l_t, in_=dist_sb[t], func=AF.Ln, bias=1.0)
                    logd_sb.append(l_t)
                for t in range(2):
                    isl = slice(t * 128, (t + 1) * 128)
                    rec = work.tile([128, NNUC], f32, tag="rec", name="rec")
                    nc.vector.reciprocal(rec, dist_sb[t])
                    # stack [g | g] and [logd | logd] so the transposed copies
                    # exist on both partition halves (DVE lanes cannot cross
                    # partitions, so each consumer needs an aligned source)
                    g2 = work.tile([128, 128], f32, tag="g2", name="g2")
                    nc.vector.tensor_mul(g2[:, 0:NNUC], logd_sb[t], rec)
                    nc.vector.tensor_copy(g2[:, NNUC:128], g2[:, 0:NNUC])
                    l2 = work.tile([128, 128], f32, tag="l2", name="l2")
                    nc.vector.tensor_copy(l2[:, 0:NNUC], logd_sb[t])
                    nc.vector.tensor_copy(l2[:, NNUC:128], logd_sb[t])
                    g2T_ps = hpsum.tile([128, 128], f32, tag="ps", name="ps")
                    nc.tensor.transpose(g2T_ps, g2, EYE_sb)
                    g2T = work.tile([128, 128], f32, tag="g2T", name="g2T")
                    nc.vector.tensor_copy(g2T, g2T_ps)
                    l2T_ps = hpsum.tile([128, 128], f32, tag="ps", name="ps")
                    nc.tensor.transpose(l2T_ps, l2, EYE_sb)
                    # embT chunk A: k=0 block rows 0:64, k=1 block rows 64:128
                    nc.vector.tensor_mul(
                        embA_sb[t][0:64, :], den_sb[0:64, isl], g2T[0:64, :]
                    )
                    nc.vector.tensor_mul(
                        embA_sb[t][64:128, :], den_sb[64:128, isl], g2T[64:128, :]
                    )
                    # embT chunk B: k=2 block rows 0:64, log-dist rows 64:128
                    nc.vector.tensor_mul(
                        embB_sb[t][0:64, :], denb_sb[:, isl], g2T[0:64, :]
                    )
                    nc.vector.tensor_copy(embB_sb[t][64:128, :], l2T_ps[64:128, :])

            # rotating rhs buffers V[b] = [r_j(3) ; B_h(1) ; 1(1)]
            NBUF = 3
            V = []
            for b in range(NBUF):
                v = const.tile([5, E], f16, tag=f"V{b}", name=f"V{b}")
                nc.sync.dma_start(out=v[0:3, :], in_=RJ4[0:3, :])
                nc.sync.dma_start(out=v[4:5, :], in_=RJ4[3:4, :])
                V.append(v)

            # ---- main loop: y = a2*z + gam via K=5 matmul; sigmoid split ----
            with tc.tile_pool(name="pa", bufs=2, space="PSUM") as pa, tc.tile_pool(
                name="pd", bufs=2, space="PSUM"
            ) as pd:
                for h in range(H):
                    b = h % NBUF
                    nc.sync.dma_start(out=V[b][3:4, :], in_=BTd[h : h + 1, :])
                    for t in range(2):
                        u = 2 * h + t
                        lt = LT_sb[0:5, u * 128 : (u + 1) * 128]
                        pa_t = pa.tile([128, JH], f32, tag="pa", name="pa")
                        pd_t = pd.tile([128, JH], f32, tag="pd", name="pd")
                        for c in range(2):
                            nc.tensor.matmul(
                                pa_t[:, c * 512 : (c + 1) * 512],
                                lt,
                                V[b][0:5, c * 512 : (c + 1) * 512],
                                start=True,
                                stop=True,
                            )
                        for c in range(2):
                            nc.tensor.matmul(
                                pd_t[:, c * 512 : (c + 1) * 512],
                                lt,
                                V[b][0:5, JH + c * 512 : JH + (c + 1) * 512],
                                start=True,
                                stop=True,
                            )
                        nc.scalar.activation(
                            out=pa_t,
                            in_=pa_t,
                            func=AF.Sigmoid,
                            bias=bia_sb[:, 0:1],
                            scale=_SCL,
                            accum_out=SA_sb[t][:, h : h + 1],
                        )
                        nc.vector._custom_dve(
                            pl5,
                            out=pd_t,
                            in0=pd_t,
                            s0=_RHO,
                            s1=_PC1,
                            imm2=_PC2,
                            accum_out=SD_sb[t][:, h : h + 1],
                        )

            # ---- tail: S = SA + SD, output projection ----
            with tc.tile_pool(name="tpsum", bufs=2, space="PSUM") as tpsum:
                for t in range(2):
                    isl = slice(t * 128, (t + 1) * 128)
                    nc.vector.tensor_add(S_sb[t], SA_sb[t], SD_sb[t])
                    ST_ps = tpsum.tile([H, 128], f32, tag="ps", name="ps")
                    nc.tensor.transpose(ST_ps, S_sb[t], EYE_sb)
                    ST_sb = work.tile([H, 128], f32, tag="ST", name="ST")
                    nc.vector.tensor_copy(ST_sb, ST_ps)
                    O_ps = tpsum.tile([128, 64], f32, tag="ps2", name="ps2")
                    nc.tensor.matmul(O_ps, ST_sb, W2A_sb, start=True, stop=False)
                    nc.tensor.matmul(O_ps, embA_sb[t], W0A_sb, start=False, stop=False)
                    nc.tensor.matmul(O_ps, embB_sb[t], W0B_sb, start=False, stop=False)
                    nc.tensor.matmul(
                        O_ps, spin1_sb[:, isl], W0C_sb, start=False, stop=True
                    )
                    O_sb = work.tile([128, 64], f32, tag="O", name="O")
                    nc.vector.tensor_copy(O_sb, O_ps)
                    nc.sync.dma_start(out=out[isl, :], in_=O_sb)

    # Raw Bass skips the Bacc codegen pass that fills `.instr` bytes for
    # InstISA subclasses (incl. InstCustomDveAnt); without it walrus fails
    # with "ISA wrong length".
    mybir.codegen_inst_isa_subclasses(nc)
    return nc


def _host_prep(r, R, W0, b0, W1s, W2s, n_up, n_down):
    r = np.asarray(r, np.float32)
    R = np.asarray(R, np.float32)
    W0 = np.asarray(W0, np.float32)
    b0 = np.asarray(b0, np.float32)
    W1s = np.asarray(W1s, np.float32)
    W2s = np.asarray(W2s, np.float32)
    n_up = int(n_up)

    W1cat = np.concatenate([W1s[0], W1s[1], W1s[2]], axis=1)  # [4, 96]
    w4 = W1cat[3]
    W2cat = np.concatenate([W2s[0], W2s[1], W2s[2]], axis=0).astype(np.float32)
    s3 = -2.0 * w4  # [H]

    n2 = (r * r).sum(1).astype(np.float32)
    # A[i,h] = r_i.w_h + |r_i|^2 w4_h ; B[j,h] = -r_j.w_h + |r_j|^2 w4_h
    rw = r @ W1cat[0:3]
    n2w4 = n2[:, None] * w4[None, :]
    Afull = (rw + n2w4).astype(np.float32)  # [E, H]
    BT = np.ascontiguousarray((-rw + n2w4).T.astype(np.float16))  # [H, E]

    RJ4 = np.concatenate(
        [r.T.astype(np.float16), np.ones((1, E), np.float16)], axis=0
    )  # [4, E]

    R2 = (R * R).sum(1).astype(np.float32)
    REN = np.concatenate(
        [-2.0 * R.T, np.ones((1, NNUC), np.float32), R2[None]], axis=0
    ).astype(np.float32)
    U5 = np.stack(
        [r[:, 0], r[:, 1], r[:, 2], n2, np.ones(E, np.float32)]
    ).astype(np.float32)

    # den[k*64+n, i] = r[i,k] - R[n,k]
    den = (r.T[:, None, :] - R.T[:, :, None]).reshape(3 * NNUC, E).astype(np.float32)

    spin = np.ones(E, np.float32)
    spin[n_up:] = -1.0
    spin1 = np.stack([spin, np.ones(E, np.float32)]).astype(np.float32)

    # permuted W0 chunks matching the on-device embT feature order
    n_idx = np.arange(NNUC)
    perm_a = np.concatenate([3 * n_idx, 3 * n_idx + 1])
    perm_b = np.concatenate([3 * n_idx + 2, 192 + n_idx])
    W0A = W0[perm_a].astype(np.float32)
    W0B = W0[perm_b].astype(np.float32)
    W0C = np.stack([W0[256], b0]).astype(np.float32)

    eye = np.eye(128, dtype=np.float32)

    shared = {
        "BTd": BT,
        "RJ4": RJ4,
        "REN": REN,
        "W2A": W2cat,
        "W0A": W0A,
        "W0B": W0B,
        "W0C": W0C,
        "EYE": eye,
    }
    in_maps = []
    for c in range(NCORES):
        isl = slice(c * EI, (c + 1) * EI)
        # lhsT table LT[:, u*128+p] for u = 2*h + t:
        #   rows 0-2: a2*s_h*r_i ; row 3: a2 ; row 4: a2*A[i,h] + gam
        rl = r[isl]  # [EI, 3]
        Al = Afull[isl]  # [EI, H]
        lt = np.empty((5, NU * 128), np.float16)
        rs = rl.T.reshape(3, 2, 128)  # [3, t, p]
        for hh in range(H):
            for t in range(2):
                u = 2 * hh + t
                lt[0:3, u * 128 : (u + 1) * 128] = (_A2 * s3[hh]) * rs[:, t, :]
                lt[3, u * 128 : (u + 1) * 128] = _A2
                lt[4, u * 128 : (u + 1) * 128] = (
                    _A2 * Al[t * 128 : (t + 1) * 128, hh] + _GAM
                )
        m = dict(shared)
        m["LTd"] = lt
        m["den"] = np.ascontiguousarray(den[:, isl])
        m["spin1"] = np.ascontiguousarray(spin1[:, isl])
        m["U5L"] = np.ascontiguousarray(U5[:, isl])
        in_maps.append(m)
    return in_maps


def _get_runner():
    """Build the Bass program once and hold a single jitted shard_map
    executable so repeat kernel() calls skip retracing/recompiling.
    Mirrors concourse.bass2jax.run_bass_via_pjrt's multi-core path."""
    if "runner" in _CACHE:
        return _CACHE["runner"]

    import jax
    from jax.experimental.shard_map import shard_map
    from jax.sharding import Mesh, PartitionSpec

    from concourse import mybir
    from concourse.bass2jax import (
        _bass_exec_p,
        install_neuronx_cc_hook,
        partition_id_tensor,
    )

    _install_compile_patch()
    install_neuronx_cc_hook()
    nc = _CACHE.setdefault("nc", _build())

    partition_name = nc.partition_id_tensor.name if nc.partition_id_tensor else None
    in_names = []
    out_names = []
    out_avals = []
    zero_outs = []
    for alloc in nc.m.functions[0].allocations:
        if not isinstance(alloc, mybir.MemoryLocationSet):
            continue
        name = alloc.memorylocations[0].name
        if alloc.kind == "ExternalInput":
            if name != partition_name:
                in_names.append(name)
        elif alloc.kind == "ExternalOutput":
            shape = tuple(alloc.tensor_shape)
            dtype = mybir.dt.np(alloc.dtype)
            out_names.append(name)
            out_avals.append(jax.core.ShapedArray(shape, dtype))
            zero_outs.append(np.zeros(shape, dtype))
    n_params = len(in_names)
    n_outs = len(out_names)
    all_in_names = list(in_names) + list(out_names)
    if partition_name is not None:
        all_in_names.append(partition_name)
    donate = tuple(range(n_params, n_params + n_outs))

    def _body(*args):
        operands = list(args)
        if partition_name is not None:
            operands.append(partition_id_tensor())
        outs = _bass_exec_p.bind(
            *operands,
            out_avals=tuple(out_avals),
            in_names=tuple(all_in_names),
            out_names=tuple(out_names),
            lowering_input_output_aliases=(),
            sim_require_finite=True,
            sim_require_nnan=True,
            nc=nc,
        )
        return tuple(outs)

    devices = jax.devices()[:NCORES]
    mesh = Mesh(np.asarray(devices), ("core",))
    in_specs = (PartitionSpec("core"),) * (n_params + n_outs)
    out_specs = (PartitionSpec("core"),) * n_outs
    sharded = jax.jit(
        shard_map(
            _body, mesh=mesh, in_specs=in_specs, out_specs=out_specs, check_rep=False
        ),
        donate_argnums=donate,
        keep_unused=True,
    )

    def runner(in_maps):
        concat_in = [
            np.concatenate([np.asarray(in_maps[c][n]) for c in range(NCORES)], axis=0)
            for n in in_names
        ]
        concat_zeros = [
            np.zeros((NCORES * z.shape[0], *z.shape[1:]), z.dtype) for z in zero_outs
        ]
        out_arrs = sharded(*concat_in, *concat_zeros)
        return np.asarray(out_arrs[out_names.index("out")])

    _CACHE["runner"] = runner
    return runner


def kernel(r, R, W0, b0, W1s, W2s, n_up, n_down):
    runner = _get_runner()
    in_maps = _host_prep(r, R, W0, b0, W1s, W2s, n_up, n_down)
    return runner(in_maps)
